# revision 15
# baseline (speedup 1.0000x reference)
"""DeformableBiomarkerAttention Trainium2 kernel (v2).

Strategy: pure data-parallel over batch (8 batches per NeuronCore, 8 cores).
v2 redesign vs baseline:
  - bf16 activations/weights on device (gate is 2e-2; measured ~5e-3)
  - sample_proj folded into Wk/Wv on host (weights-only fusion); k-bias
    dropped (softmax shift invariance), v-bias folded into out bias
  - scores via u = blockdiag(Wk'^T) q (free dim 96 instead of 256);
    ctx via per-head attn-weighted sums S of raw sampled rows
  - trilinear combine on PE: psum-accumulated matmuls against diag(w)
  - two-wave (4-batch) pipeline so writes overlap compute
  - output written as broadcast-source DMAs (one big DMA per batch)
"""

import numpy as np
import ml_dtypes

import concourse.bass as bass
import concourse.mybir as mybir
import concourse.tile as tile
from concourse import bass_utils

F32 = mybir.dt.float32
BF16 = mybir.dt.bfloat16
I32 = mybir.dt.int32
ALU = mybir.AluOpType
ACTF = mybir.ActivationFunctionType
AX = mybir.AxisListType

E = 768
CH = 6            # number of 128-channel chunks
NB = 32           # points per batch
BPC = 8           # batches per core
FULLN = 513
NCORES = 8
ROWS = BPC * NB   # 256 sampled rows per core
NG = 2            # waves / partition groups of 128 rows (4 batches each)
NH = 12           # heads
HD = 64           # head dim

# consF (f32) column layout
CF_BASE = 0       # [128, 6]  base coords (g0 xyz, g1 xyz)
CF_OFFS = 6       # [128, 6]  offsets
CF_ROWB = 12      # [128, 2]  rowbase per group
CF_MUL3 = 14      # [128, 6]  (1, 8, 64, 1, 8, 64)
CF_BQ = 20        # [128, 6]  q bias (pre-scaled by 1/8), chunked
CF_HSEL = 26      # [128, 72] head-select mask per chunk
CF_CONF = 98      # [4, 2]   confidence: [p, g] = conf[4g+p]
CF_W = 100

# consB (bf16) column layout
CB_BIO = 0        # [128, 48]  bio_embed^T chunked: [:, 8*ci + b]
CB_IDEN = 48      # [128, 128] identity
CB_ONEH = 176     # [4, 512]   onehL[j, 128j:128j+128] = 1
CB_W = 688


def _body(ctx, tc):
    nc = tc.nc

    x = nc.dram_tensor("x", [BPC * FULLN, E], BF16, kind="ExternalInput").ap()
    consF = nc.dram_tensor("consF", [128, CF_W], F32, kind="ExternalInput").ap()
    consB = nc.dram_tensor("consB", [128, CB_W], BF16, kind="ExternalInput").ap()
    bo_bc = nc.dram_tensor("bo_bc", [4, NG * E], F32, kind="ExternalInput").ap()
    wqt = nc.dram_tensor("wqt", [128, CH, E], BF16, kind="ExternalInput").ap()
    wkn = nc.dram_tensor("wkn", [128, CH, E], BF16, kind="ExternalInput").ap()
    wvt = nc.dram_tensor("wvt", [128, CH, E], BF16, kind="ExternalInput").ap()
    wot = nc.dram_tensor("wot", [128, CH, E], BF16, kind="ExternalInput").ap()
    out = nc.dram_tensor("out", [BPC * FULLN, E], F32, kind="ExternalOutput").ap()

    cpool = ctx.enter_context(tc.tile_pool(name="consts", bufs=1))
    gpool = ctx.enter_context(tc.tile_pool(name="gather", bufs=5))
    tpool = ctx.enter_context(tc.tile_pool(name="tmp", bufs=3))
    btpool = ctx.enter_context(tc.tile_pool(name="bt", bufs=3))
    pp = ctx.enter_context(tc.tile_pool(name="ps", bufs=6, space="PSUM"))

    _n = [0]

    def psum(shape, dt=F32):
        _n[0] += 1
        return pp.tile(shape, dt, tag="ps", name=f"ps{_n[0]}")

    def ctile(shape, dt=F32, tag=None):
        _n[0] += 1
        tag = tag or f"c{_n[0]}"
        return cpool.tile(shape, dt, tag=tag, name=tag)

    # ---- input DMAs (SP queue) ----
    cf = ctile([128, CF_W], tag="consF")
    nc.sync.dma_start(out=cf[:], in_=consF[:])
    cb = ctile([128, CB_W], BF16, tag="consB")
    nc.sync.dma_start(out=cb[:], in_=consB[:])
    bo_t = ctile([4, NG * E], tag="bo")
    nc.sync.dma_start(out=bo_t[:], in_=bo_bc[:])
    w_t = {}
    for name, ap in (("wqt", wqt), ("wkn", wkn), ("wvt", wvt), ("wot", wot)):
        t = cpool.tile([128, CH, E], BF16, tag=name)
        nc.sync.dma_start(out=t[:], in_=ap[:])
        w_t[name] = t

    iden = cb[:, CB_IDEN:CB_IDEN + 128]
    hsel = cf[:, CF_HSEL:CF_HSEL + CH * NH]

    # ---- coords -> corner indices + trilinear weights (both groups, [128,6])
    c_t = ctile([128, 6], tag="c")
    nc.vector.tensor_add(out=c_t[:], in0=cf[:, CF_BASE:CF_BASE + 6],
                         in1=cf[:, CF_OFFS:CF_OFFS + 6])
    i_t = ctile([128, 6], tag="i")
    nc.vector.tensor_scalar(out=i_t[:], in0=c_t[:], scalar1=1.0, scalar2=-1.0,
                            op0=ALU.min, op1=ALU.max)
    nc.vector.tensor_scalar(out=i_t[:], in0=i_t[:], scalar1=1.0, scalar2=3.5,
                            op0=ALU.add, op1=ALU.mult)
    ri_t = ctile([128, 6], I32, tag="ri")
    nc.vector.tensor_copy(out=ri_t[:], in_=i_t[:])
    rf_t = ctile([128, 6], tag="rf")
    nc.vector.tensor_copy(out=rf_t[:], in_=ri_t[:])
    neg_t = ctile([128, 6], tag="neg")
    nc.vector.tensor_tensor(out=neg_t[:], in0=i_t[:], in1=rf_t[:], op=ALU.is_lt)
    i0_t = ctile([128, 6], tag="i0")
    nc.vector.tensor_sub(out=i0_t[:], in0=rf_t[:], in1=neg_t[:])
    nc.vector.tensor_scalar(out=i0_t[:], in0=i0_t[:], scalar1=6.0, scalar2=None,
                            op0=ALU.min)
    w_tt = ctile([128, 6], tag="w")
    nc.vector.tensor_sub(out=w_tt[:], in0=i_t[:], in1=i0_t[:])
    omw_t = ctile([128, 6], tag="omw")
    nc.vector.tensor_scalar(out=omw_t[:], in0=w_tt[:], scalar1=-1.0, scalar2=1.0,
                            op0=ALU.mult, op1=ALU.add)
    pr_t = ctile([128, 6], tag="pr")
    nc.vector.tensor_mul(out=pr_t[:], in0=i0_t[:],
                         in1=cf[:, CF_MUL3:CF_MUL3 + 6])
    ib_t = ctile([128, 2], tag="ib")
    nc.vector.tensor_reduce(out=ib_t[:], in_=pr_t[:].rearrange("p (g k) -> p g k", k=3),
                            axis=AX.X, op=ALU.add)
    nc.vector.tensor_add(out=ib_t[:], in0=ib_t[:],
                         in1=cf[:, CF_ROWB:CF_ROWB + 2])

    w3 = w_tt[:].rearrange("p (g k) -> p g k", k=3)
    omw3 = omw_t[:].rearrange("p (g k) -> p g k", k=3)
    wyz_t = ctile([128, 2, 4], tag="wyz")
    wc_t = ctile([128, 2, 8], tag="wc")
    idxf_t = ctile([128, 2, 4], tag="idxf")
    for j, (cz, cy) in enumerate(((0, 0), (0, 1), (1, 0), (1, 1))):
        ysel = w3[:, :, 1:2] if cy else omw3[:, :, 1:2]
        zsel = w3[:, :, 2:3] if cz else omw3[:, :, 2:3]
        nc.vector.tensor_mul(out=wyz_t[:, :, j:j + 1], in0=ysel, in1=zsel)
        nc.vector.tensor_mul(out=wc_t[:, :, 2 * j:2 * j + 1],
                             in0=wyz_t[:, :, j:j + 1], in1=omw3[:, :, 0:1])
        nc.vector.tensor_mul(out=wc_t[:, :, 2 * j + 1:2 * j + 2],
                             in0=wyz_t[:, :, j:j + 1], in1=w3[:, :, 0:1])
        nc.vector.tensor_scalar(out=idxf_t[:, :, j:j + 1], in0=ib_t[:].unsqueeze(2),
                                scalar1=float(64 * cz + 8 * cy), scalar2=None,
                                op0=ALU.add)
    idx_t = ctile([128, 2, 4], I32, tag="idx")
    nc.vector.tensor_copy(out=idx_t[:], in_=idxf_t[:])

    # bopc[p, g, :] = out bias * confidence[4g+p] (off critical path)
    bopc = ctile([4, NG, E], tag="bopc")
    for g in range(NG):
        nc.vector.tensor_scalar(out=bopc[:, g, :], in0=bo_t[:, E * g:E * (g + 1)],
                                scalar1=cf[0:4, CF_CONF + g:CF_CONF + g + 1],
                                scalar2=None, op0=ALU.mult)

    # diag(wc) matrices, bf16 [128, 128] per (group, corner)
    diag = {}
    for g in range(NG):
        for c8 in range(8):
            d = ctile([128, 128], BF16, tag=f"diag{g}_{c8}")
            nc.vector.tensor_scalar(out=d[:], in0=iden,
                                    scalar1=wc_t[:, g, c8:c8 + 1], scalar2=None,
                                    op0=ALU.mult)
            diag[(g, c8)] = d

    # ---- gathers: 4 paired-corner indirect DMAs per group ----
    corner = {}
    for g in range(NG):
        for j in range(4):
            t = gpool.tile([128, 2 * E], BF16, tag="corner", name=f"cr{g}_{j}")
            nc.gpsimd.indirect_dma_start(
                out=t[:], out_offset=None, in_=x[:],
                in_offset=bass.IndirectOffsetOnAxis(ap=idx_t[:, g, j:j + 1],
                                                    axis=0))
            corner[(g, j)] = t

    # ---- q projection: qT[128, CH, 8] bf16, pre-scaled by 1/8 ----
    q_ps = psum([128, CH, BPC])
    for co in range(CH):
        for ci in range(CH):
            nc.tensor.matmul(out=q_ps[:, co, :],
                             lhsT=w_t["wqt"][:, ci, 128 * co:128 * (co + 1)],
                             rhs=cb[:, CB_BIO + 8 * ci:CB_BIO + 8 * (ci + 1)],
                             start=(ci == 0), stop=(ci == CH - 1))
    qT = ctile([128, CH, BPC], BF16, tag="qT")
    for co in range(CH):
        nc.scalar.activation(out=qT[:, co, :], in_=q_ps[:, co, :],
                             func=ACTF.Identity,
                             bias=cf[:, CF_BQ + co:CF_BQ + co + 1], scale=0.125)

    # qmask: [128, 96] per chunk (columns = 12*b + h)
    qm = []
    for ci in range(CH):
        t = ctile([128, BPC, NH], BF16, tag=f"qm{ci}")
        nc.vector.tensor_mul(
            out=t[:],
            in0=qT[:, ci, :].unsqueeze(2).to_broadcast([128, BPC, NH]),
            in1=hsel[:, NH * ci:NH * (ci + 1)].unsqueeze(1)
                .to_broadcast([128, BPC, NH]))
        qm.append(t)

    # ---- trilinear combine on PE: natural layout [128, 768] per group ----
    samp_nat = []
    for g in range(NG):
        nat = ctile([128, E], BF16, tag=f"nat{g}")
        for half in range(2):
            ps = psum([128, 384])
            for c8 in range(8):
                j, xb = divmod(c8, 2)
                nc.tensor.matmul(
                    out=ps[:], lhsT=diag[(g, c8)][:],
                    rhs=corner[(g, j)][:, E * xb + 384 * half:
                                       E * xb + 384 * (half + 1)],
                    start=(c8 == 0), stop=(c8 == 7))
            nc.scalar.copy(out=nat[:, 384 * half:384 * (half + 1)], in_=ps[:])
        samp_nat.append(nat)

    # ---- transpose sampled -> sampT (channel-on-partition), bf16 ----
    sampT = [ctile([128, ROWS], BF16, tag=f"sampT{c}") for c in range(CH)]
    for g in range(NG):
        for chk in range(CH):
            ps = psum([128, 128], BF16)
            nc.tensor.transpose(out=ps[:],
                                in_=samp_nat[g][:, 128 * chk:128 * (chk + 1)],
                                identity=iden)
            nc.scalar.copy(out=sampT[chk][:, 128 * g:128 * (g + 1)], in_=ps[:])

    # ---- u = blockdiag(Wk'^T) q : [128, 96] per chunk ----
    u_sb = []
    for co in range(CH):
        ps = psum([128, BPC * NH])
        for ci in range(CH):
            nc.tensor.matmul(out=ps[:],
                             lhsT=w_t["wkn"][:, ci, 128 * co:128 * (co + 1)],
                             rhs=qm[ci][:], start=(ci == 0), stop=(ci == CH - 1))
        t = ctile([128, BPC * NH], BF16, tag=f"u{co}")
        nc.scalar.copy(out=t[:], in_=ps[:])
        u_sb.append(t)

    # ---- per-wave attention ----
    ctxT = ctile([128, CH, BPC], BF16, tag="ctxT")
    outfin = [ctile([4, E], BF16, tag=f"of{g}") for g in range(NG)]
    S_sb = [ctile([128, BPC * NH], BF16, tag=f"S{c}") for c in range(CH)]
    attn_blk = []
    for g in range(NG):
        t = ctile([128, 4 * NH], BF16, tag=f"ablk{g}")
        nc.vector.memset(t[:], 0)
        attn_blk.append(t)

    def wave(g):
        # scores [12, 4, 32]
        sc_ps = psum([NH, 4, NB])
        for bl in range(4):
            b = 4 * g + bl
            for ci in range(CH):
                nc.tensor.matmul(
                    out=sc_ps[:, bl, :],
                    lhsT=u_sb[ci][:, NH * b:NH * (b + 1)],
                    rhs=sampT[ci][:, 128 * g + NB * bl:128 * g + NB * (bl + 1)],
                    start=(ci == 0), stop=(ci == CH - 1))
        # softmax over points (no max-sub; logits are small by construction)
        ex = tpool.tile([NH, 4, NB], F32, tag="ex", name=f"ex{g}")
        nc.scalar.activation(out=ex[:], in_=sc_ps[:], func=ACTF.Exp)
        sm = tpool.tile([NH, 4, 1], F32, tag="sm", name=f"sm{g}")
        nc.vector.tensor_reduce(out=sm[:], in_=ex[:], axis=AX.X, op=ALU.add)
        rc = tpool.tile([NH, 4, 1], F32, tag="rc", name=f"rc{g}")
        nc.vector.reciprocal(out=rc[:], in_=sm[:])
        at = tpool.tile([NH, 4, NB], BF16, tag="at", name=f"at{g}")
        nc.vector.tensor_mul(out=at[:], in0=ex[:],
                             in1=rc[:].to_broadcast([NH, 4, NB]))
        # attn -> block-diagonal [128, 48] (transpose per batch; the psum
        # write lands at partition block 32*bl so the sbuf copy is
        # partition-aligned)
        at_ps = psum([128, NH], BF16)
        for bl in range(4):
            nc.tensor.transpose(out=at_ps[NB * bl:NB * (bl + 1), :],
                                in_=at[:, bl, :], identity=iden[0:NH, 0:NH],
                                tile_position=(0, NB * bl))
            nc.scalar.copy(
                out=attn_blk[g][NB * bl:NB * (bl + 1), NH * bl:NH * (bl + 1)],
                in_=at_ps[NB * bl:NB * (bl + 1), :])
        # S = samp^T @ attn_blk : [128, 48] per chunk -> S_sb cols 48g:+48
        for chk in range(CH):
            ps = psum([128, 4 * NH])
            nc.tensor.matmul(out=ps[:],
                             lhsT=samp_nat[g][:, 128 * chk:128 * (chk + 1)],
                             rhs=attn_blk[g][:], start=True, stop=True)
            nc.scalar.copy(out=S_sb[chk][:, 48 * g:48 * (g + 1)], in_=ps[:])
        # ctx = blockdiag(Wv') S, via full product + head select
        for co in range(CH):
            ps = psum([128, 4 * NH])
            for ci in range(CH):
                nc.tensor.matmul(out=ps[:],
                                 lhsT=w_t["wvt"][:, ci, 128 * co:128 * (co + 1)],
                                 rhs=S_sb[ci][:, 48 * g:48 * (g + 1)],
                                 start=(ci == 0), stop=(ci == CH - 1))
            prod = tpool.tile([128, 4, NH], F32, tag="prod", name=f"pr{g}_{co}")
            nc.vector.tensor_mul(
                out=prod[:],
                in0=ps[:].rearrange("p (b h) -> p b h", h=NH),
                in1=hsel[:, NH * co:NH * (co + 1)].unsqueeze(1)
                    .to_broadcast([128, 4, NH]))
            with nc.allow_low_precision(reason="12-term head-select sum; "
                                        "bf16 out is within error budget"):
                nc.vector.tensor_reduce(out=ctxT[:, co, 4 * g:4 * (g + 1)],
                                        in_=prod[:], axis=AX.X, op=ALU.add)
        # out projection + bias + confidence -> outfin[g] (bf16, parts 0-3)
        for half in range(2):
            sl = slice(384 * half, 384 * (half + 1))
            ps = psum([4, 384])
            for ci in range(CH):
                nc.tensor.matmul(out=ps[:], lhsT=ctxT[:, ci, 4 * g:4 * (g + 1)],
                                 rhs=w_t["wot"][:, ci, sl],
                                 start=(ci == 0), stop=(ci == CH - 1))
            nc.vector.scalar_tensor_tensor(
                out=outfin[g][:, sl], in0=ps[:],
                scalar=cf[0:4, CF_CONF + g:CF_CONF + g + 1],
                in1=bopc[:, g, sl], op0=ALU.mult, op1=ALU.add)

    def write_batch(b):
        g, bl = divmod(b, 4)
        bt = btpool.tile([128, E], F32, tag="bt", name=f"bt{b}")
        for half in range(2):
            sl = slice(384 * half, 384 * (half + 1))
            ps = psum([128, 384])
            nc.tensor.matmul(
                out=ps[:],
                lhsT=cb[0:4, CB_ONEH + 128 * bl:CB_ONEH + 128 * (bl + 1)],
                rhs=outfin[g][:, sl], start=True, stop=True)
            nc.scalar.copy(out=bt[:, sl], in_=ps[:])
        r0 = FULLN * b
        nc.sync.dma_start(
            out=out[r0:r0 + 512, :].rearrange("(i p) c -> p i c", p=128),
            in_=bt[:].unsqueeze(1).to_broadcast([128, 4, E]))
        nc.sync.dma_start(out=out[r0 + 512:r0 + 513, :], in_=bt[0:1, :])

    wave(0)
    for b in range(4):
        write_batch(b)
    wave(1)
    for b in range(4, 8):
        write_batch(b)


_NO_SPLIT_TYPES = {"InstUnconditionalBranch", "InstConditionalBranch"}


def _split_waits(nc, max_waits=1):
    # walrus (CoreV3) accepts only one sync-wait command per compute
    # instruction; move extra waits onto injected same-engine NoOps placed
    # immediately before the instruction (semantics unchanged).
    import bass_rust
    k = 0
    for fn in nc.m.functions:
        for bb in fn.blocks:
            insts = bb.instructions
            i = 0
            while i < len(insts):
                inst = insts[i]
                si = inst.sync_info
                if (type(inst).__name__ not in _NO_SPLIT_TYPES
                        and si is not None
                        and si.on_wait and len(si.on_wait) > max_waits):
                    waits = list(si.on_wait)
                    extra, keep = waits[:-max_waits], waits[-max_waits:]
                    for w in extra:
                        k += 1
                        nop = bass_rust.InstNoOp(name=f"I-wsplit-{k}",
                                                 engine=inst.engine,
                                                 ins=[], outs=[])
                        nop.sync_info = bass_rust.SyncInfo(on_wait=[w],
                                                           on_update=[])
                        insts.insert(i, nop)
                        i += 1
                    inst.sync_info = bass_rust.SyncInfo(
                        on_wait=keep, on_update=list(si.on_update or []))
                i += 1
    return k


def build(split=True):
    from contextlib import ExitStack

    nc = bass.Bass("TRN2", debug=False, num_devices=NCORES)
    with tile.TileContext(nc) as tc, ExitStack() as es:
        _body(es, tc)
    if split:
        # needed for the walrus compile; CoreSim can't replay injected nops
        _split_waits(nc)
    return nc


def host_prep(inputs):
    """Build per-core in_maps from full inputs (layout/dtype marshalling +
    weights-only fusion)."""
    BF = ml_dtypes.bfloat16
    x = np.asarray(inputs["x"], dtype=np.float32)
    bio = np.asarray(inputs["bio_embed"], dtype=np.float32)
    base = np.asarray(inputs["base_coords"], dtype=np.float32)
    offsets = np.asarray(inputs["offsets"], dtype=np.float32)
    confidence = np.asarray(inputs["confidence"], dtype=np.float32)
    wsp = np.asarray(inputs["sample_proj_w"], dtype=np.float32)
    bsp = np.asarray(inputs["sample_proj_b"], dtype=np.float32)
    win = np.asarray(inputs["in_proj_w"], dtype=np.float32)
    bin_ = np.asarray(inputs["in_proj_b"], dtype=np.float32)
    wout = np.asarray(inputs["out_proj_w"], dtype=np.float32)
    bout = np.asarray(inputs["out_proj_b"], dtype=np.float32)

    # weights-only fusion
    wq = win[:E]
    wkp = win[E:2 * E] @ wsp
    wvp = win[2 * E:] @ wsp
    bop = wout @ (win[2 * E:] @ bsp + bin_[2 * E:]) + bout

    def chunkT(w):  # [E, E] -> [128, CH, E] of w^T (bf16)
        return np.ascontiguousarray(
            w.T.reshape(CH, 128, E).transpose(1, 0, 2)).astype(BF)

    def chunkN(w):  # [E, E] -> [128, CH, E] natural rows (bf16)
        return np.ascontiguousarray(
            w.reshape(CH, 128, E).transpose(1, 0, 2)).astype(BF)

    consF = np.zeros((128, CF_W), np.float32)
    consF[:, CF_BASE:CF_BASE + 6] = np.tile(np.tile(base, (4, 1)), (1, 2)) \
        .reshape(128, 6)
    consF[:, CF_ROWB:CF_ROWB + 2] = (
        (np.arange(ROWS) // NB) * FULLN + 1.0).astype(np.float32) \
        .reshape(2, 128).T
    consF[:, CF_MUL3:CF_MUL3 + 6] = np.tile(
        np.array([1.0, 8.0, 64.0], np.float32), (128, 2))
    consF[:, CF_BQ:CF_BQ + CH] = (bin_[:E] * 0.125).reshape(CH, 128).T
    hsel = np.zeros((128, CH, NH), np.float32)
    for ci in range(CH):
        for p in range(128):
            hsel[p, ci, (ci * 128 + p) // HD] = 1.0
    consF[:, CF_HSEL:CF_HSEL + CH * NH] = hsel.reshape(128, CH * NH)

    consB = np.zeros((128, CB_W), np.float32)
    consB[:, CB_IDEN:CB_IDEN + 128] = np.eye(128, dtype=np.float32)
    for j in range(4):
        consB[j, CB_ONEH + 128 * j:CB_ONEH + 128 * (j + 1)] = 1.0

    shared = {
        "wqt": chunkT(wq),
        "wkn": chunkN(wkp),
        "wvt": chunkT(wvp),
        "wot": chunkT(wout),
    }
    bo_full = np.tile(bop[None, :], (4, NG)).astype(np.float32)  # [4, 2E]

    x16 = x.astype(BF)
    in_maps = []
    for c in range(NCORES):
        bsl = slice(BPC * c, BPC * (c + 1))
        cF = consF.copy()
        # offsets per row: row r of group g = batch 4g + r//32, point r%32
        offs_c = offsets[bsl].reshape(NG, 128, 3)
        cF[:, CF_OFFS:CF_OFFS + 3] = offs_c[0]
        cF[:, CF_OFFS + 3:CF_OFFS + 6] = offs_c[1]
        cF[0:4, CF_CONF:CF_CONF + NG] = confidence[bsl, 0].reshape(NG, 4).T
        cB = consB.copy()
        cB[:, CB_BIO:CB_BIO + CH * BPC] = np.ascontiguousarray(
            bio[bsl].T.reshape(CH, 128, BPC).transpose(1, 0, 2)) \
            .reshape(128, CH * BPC)
        m = dict(shared)
        m["x"] = x16[bsl].reshape(BPC * FULLN, E)
        m["consF"] = cF
        m["consB"] = cB.astype(BF)
        m["bo_bc"] = bo_full
        in_maps.append(m)
    return in_maps


_NC = None


def kernel(**inputs):
    global _NC
    if _NC is None:
        _NC = build()
    in_maps = host_prep(inputs)
    res = bass_utils.run_bass_kernel_spmd(_NC, in_maps,
                                          core_ids=list(range(NCORES)))
    outs = [res.results[c]["out"].reshape(BPC, FULLN, E)
            for c in range(NCORES)]
    return np.concatenate(outs, axis=0)


# revision 49
# speedup vs baseline: 1.4406x; 1.4406x over previous
"""DeformableBiomarkerAttention Trainium2 kernel (v2).

Strategy: pure data-parallel over batch (8 batches per NeuronCore, 8 cores).
v2 redesign vs baseline:
  - bf16 activations/weights on device (gate is 2e-2; measured ~5e-3)
  - sample_proj folded into Wk/Wv on host (weights-only fusion); k-bias
    dropped (softmax shift invariance), v-bias folded into out bias
  - scores via u = blockdiag(Wk'^T) q (free dim 96 instead of 256);
    ctx via per-head attn-weighted sums S of raw sampled rows
  - trilinear combine on PE: psum-accumulated matmuls against diag(w)
  - two-wave (4-batch) pipeline so writes overlap compute
  - output written as broadcast-source DMAs (one big DMA per batch)
"""

import numpy as np
import ml_dtypes

import concourse.bass as bass
import concourse.mybir as mybir
import concourse.tile as tile
from concourse import bass_utils

F32 = mybir.dt.float32
BF16 = mybir.dt.bfloat16
FP8 = mybir.dt.float8e4
I32 = mybir.dt.int32
ALU = mybir.AluOpType
ACTF = mybir.ActivationFunctionType
AX = mybir.AxisListType

E = 768
CH = 6            # number of 128-channel chunks
NB = 32           # points per batch
BPC = 8           # batches per core
FULLN = 513
NCORES = 8
ROWS = BPC * NB   # 256 sampled rows per core
NG = 2            # waves / partition groups of 128 rows (4 batches each)
NH = 12           # heads
HD = 64           # head dim

# consF (f32) column layout
CF_BASE = 0       # [128, 6]  base coords (g0 xyz, g1 xyz)
CF_OFFS = 6       # [128, 6]  offsets
CF_ROWB = 12      # [128, 2]  rowbase per group
CF_MUL3 = 14      # [128, 6]  (1, 8, 64, 1, 8, 64)
CF_BQ = 20        # [128, 6]  q bias (pre-scaled by 1/8), chunked
CF_HSEL = 26      # [128, 72] head-select mask per chunk
CF_CONF = 98      # [4, 2]   confidence: [p, g] = conf[4g+p]
CF_W = 100

# consB (bf16) column layout
CB_BIO = 0        # [128, 48]  bio_embed^T chunked: [:, 8*ci + b]
CB_IDEN = 48      # [128, 128] identity
CB_BSEL = 176     # [12, 768]  bsel[h, c] = 1 if head(c) == h
CB_ONEH = 944     # [4, 512]   onehL[j, 128j:128j+128] = 1
CB_W = 1456


def _body(ctx, tc):
    nc = tc.nc

    x = nc.dram_tensor("x", [BPC * FULLN, E], BF16, kind="ExternalInput").ap()
    consF = nc.dram_tensor("consF", [128, CF_W], F32, kind="ExternalInput").ap()
    consB = nc.dram_tensor("consB", [128, CB_W], BF16, kind="ExternalInput").ap()
    bo_bc = nc.dram_tensor("bo_bc", [1, NG * E], BF16, kind="ExternalInput").ap()
    wqt = nc.dram_tensor("wqt", [128, CH, E], FP8, kind="ExternalInput").ap()
    wkn = nc.dram_tensor("wkn", [128, CH, E], FP8, kind="ExternalInput").ap()
    wvt = nc.dram_tensor("wvt", [128, CH, E], BF16, kind="ExternalInput").ap()
    wot = nc.dram_tensor("wot", [128, CH, E], BF16, kind="ExternalInput").ap()
    out = nc.dram_tensor("out", [BPC * FULLN, E], F32, kind="ExternalOutput").ap()

    cpool = ctx.enter_context(tc.tile_pool(name="consts", bufs=1))
    gpool = ctx.enter_context(tc.tile_pool(name="gather", bufs=8))
    tpool = ctx.enter_context(tc.tile_pool(name="tmp", bufs=3))
    btpool = ctx.enter_context(tc.tile_pool(name="bt", bufs=3))
    pp = ctx.enter_context(tc.tile_pool(name="ps", bufs=4, space="PSUM"))
    abcp = ctx.enter_context(tc.tile_pool(name="abc", bufs=2, space="PSUM"))

    _n = [0]

    def psum(shape, dt=F32):
        _n[0] += 1
        return pp.tile(shape, dt, tag="ps", name=f"ps{_n[0]}")

    def ctile(shape, dt=F32, tag=None):
        _n[0] += 1
        tag = tag or f"c{_n[0]}"
        return cpool.tile(shape, dt, tag=tag, name=tag)

    # ---- input DMAs (SP queue) ----
    cf = ctile([128, CF_W], tag="consF")
    nc.sync.dma_start(out=cf[:], in_=consF[:])
    cb = ctile([128, CB_W], BF16, tag="consB")
    nc.sync.dma_start(out=cb[:], in_=consB[:])
    bo_t = ctile([1, NG * E], BF16, tag="bo")
    nc.sync.dma_start(out=bo_t[:], in_=bo_bc[:])
    w_t = {}
    for name, ap in (("wqt", wqt), ("wkn", wkn)):
        t = cpool.tile([128, CH, E], FP8, tag=name)
        nc.sync.dma_start(out=t[:], in_=ap[:])
        w_t[name] = t

    iden = cb[:, CB_IDEN:CB_IDEN + 128]
    hsel = cf[:, CF_HSEL:CF_HSEL + CH * NH]

    # ---- coords -> corner indices + trilinear weights (both groups, [128,6])
    c_t = ctile([128, 6], tag="c")
    nc.vector.tensor_add(out=c_t[:], in0=cf[:, CF_BASE:CF_BASE + 6],
                         in1=cf[:, CF_OFFS:CF_OFFS + 6])
    i_t = ctile([128, 6], tag="i")
    nc.vector.tensor_scalar(out=i_t[:], in0=c_t[:], scalar1=1.0, scalar2=-1.0,
                            op0=ALU.min, op1=ALU.max)
    nc.vector.tensor_scalar(out=i_t[:], in0=i_t[:], scalar1=1.0, scalar2=3.5,
                            op0=ALU.add, op1=ALU.mult)
    ri_t = ctile([128, 6], I32, tag="ri")
    nc.vector.tensor_copy(out=ri_t[:], in_=i_t[:])
    rf_t = ctile([128, 6], tag="rf")
    nc.vector.tensor_copy(out=rf_t[:], in_=ri_t[:])
    neg_t = ctile([128, 6], tag="neg")
    nc.vector.tensor_tensor(out=neg_t[:], in0=i_t[:], in1=rf_t[:], op=ALU.is_lt)
    i0_t = ctile([128, 6], tag="i0")
    nc.vector.tensor_sub(out=i0_t[:], in0=rf_t[:], in1=neg_t[:])
    nc.vector.tensor_scalar(out=i0_t[:], in0=i0_t[:], scalar1=6.0, scalar2=None,
                            op0=ALU.min)
    w_tt = ctile([128, 6], tag="w")
    nc.vector.tensor_sub(out=w_tt[:], in0=i_t[:], in1=i0_t[:])
    omw_t = ctile([128, 6], tag="omw")
    nc.vector.tensor_scalar(out=omw_t[:], in0=w_tt[:], scalar1=-1.0, scalar2=1.0,
                            op0=ALU.mult, op1=ALU.add)
    pr_t = ctile([128, 6], tag="pr")
    nc.vector.tensor_mul(out=pr_t[:], in0=i0_t[:],
                         in1=cf[:, CF_MUL3:CF_MUL3 + 6])
    ib_t = ctile([128, 2], tag="ib")
    nc.vector.tensor_reduce(out=ib_t[:], in_=pr_t[:].rearrange("p (g k) -> p g k", k=3),
                            axis=AX.X, op=ALU.add)
    nc.vector.tensor_add(out=ib_t[:], in0=ib_t[:],
                         in1=cf[:, CF_ROWB:CF_ROWB + 2])

    w3 = w_tt[:].rearrange("p (g k) -> p g k", k=3)
    omw3 = omw_t[:].rearrange("p (g k) -> p g k", k=3)
    wyz_t = ctile([128, 2, 4], tag="wyz")
    wc_t = ctile([128, 2, 8], tag="wc")
    idxf_t = ctile([128, 2, 4], tag="idxf")
    for j, (cz, cy) in enumerate(((0, 0), (0, 1), (1, 0), (1, 1))):
        ysel = w3[:, :, 1:2] if cy else omw3[:, :, 1:2]
        zsel = w3[:, :, 2:3] if cz else omw3[:, :, 2:3]
        nc.vector.tensor_mul(out=wyz_t[:, :, j:j + 1], in0=ysel, in1=zsel)
        nc.vector.tensor_mul(out=wc_t[:, :, 2 * j:2 * j + 1],
                             in0=wyz_t[:, :, j:j + 1], in1=omw3[:, :, 0:1])
        nc.vector.tensor_mul(out=wc_t[:, :, 2 * j + 1:2 * j + 2],
                             in0=wyz_t[:, :, j:j + 1], in1=w3[:, :, 0:1])
        nc.vector.tensor_scalar(out=idxf_t[:, :, j:j + 1], in0=ib_t[:].unsqueeze(2),
                                scalar1=float(64 * cz + 8 * cy), scalar2=None,
                                op0=ALU.add)
    idx_t = ctile([128, 2, 4], I32, tag="idx")
    nc.vector.tensor_copy(out=idx_t[:], in_=idxf_t[:])

    # diag(wc) matrices, bf16 [128, 128] per (group, corner); bopc is the
    # conf-scaled out bias.  Emission of g1 diags / bopc is deferred so the
    # DVE reaches the qm ops (which gate the wv/wo loads) early.
    diag = {}

    def diag_mms(g):
        for c8 in range(8):
            d = ctile([128, 128], BF16, tag=f"diag{g}_{c8}")
            nc.vector.tensor_scalar(out=d[:], in0=iden,
                                    scalar1=wc_t[:, g, c8:c8 + 1], scalar2=None,
                                    op0=ALU.mult)
            diag[(g, c8)] = d


    diag_mms(0)

    # ---- gathers: 4 paired-corner indirect DMAs per group ----
    # (group 1 is deferred below so its DMA-queue slot comes after the
    # wv/wo weight loads; see the dep wiring near tr_mms(0))
    corner = {}
    gather_dmas = {}
    for g in range(NG):
        for j in range(4):
            t = gpool.tile([128, 2 * E], BF16, tag="corner", name=f"cr{g}_{j}")
            gather_dmas[(g, j)] = nc.gpsimd.indirect_dma_start(
                out=t[:], out_offset=None, in_=x[:],
                in_offset=bass.IndirectOffsetOnAxis(ap=idx_t[:, g, j:j + 1],
                                                    axis=0))
            corner[(g, j)] = t

    # ---- q projection: qT[128, CH, 8] bf16, pre-scaled by 1/8 ----
    q_ps = psum([128, CH, BPC])
    for co in range(CH):
        for ci in range(CH):
            nc.tensor.matmul(out=q_ps[:, co, :],
                             lhsT=w_t["wqt"][:, ci, 128 * co:128 * (co + 1)],
                             rhs=cb[:, CB_BIO + 8 * ci:CB_BIO + 8 * (ci + 1)],
                             start=(ci == 0), stop=(ci == CH - 1))
    qT = ctile([128, CH, BPC], BF16, tag="qT")
    qT_ops = []
    for co in range(CH):
        qT_ops.append(nc.scalar.activation(
            out=qT[:, co, :], in_=q_ps[:, co, :], func=ACTF.Identity,
            bias=cf[:, CF_BQ + co:CF_BQ + co + 1], scale=0.125))

    # qmask: [128, 96] per chunk (columns = 12*b + h)
    qm = []
    qm_ops = []
    for ci in range(CH):
        t = ctile([128, BPC, NH], BF16, tag=f"qm{ci}")
        qm_ops.append(nc.vector.tensor_mul(
            out=t[:],
            in0=qT[:, ci, :].unsqueeze(2).to_broadcast([128, BPC, NH]),
            in1=hsel[:, NH * ci:NH * (ci + 1)].unsqueeze(1)
                .to_broadcast([128, BPC, NH])))
        qm.append(t)

    # ---- trilinear combine on PE, directly in transposed layout:
    # sampT[c, row] accumulates corner_chunk^T @ diag(w) over the 8 corners,
    # starting as soon as each gathered pair lands ----
    sampT = [ctile([128, ROWS], BF16, tag=f"sampT{c}") for c in range(CH)]

    def tr_mms(g):
        ops = []
        for half in range(2):
            ps = psum([128, 3, 128])
            for k in range(3):
                chk = 3 * half + k
                for c8 in range(8):
                    j, xb = divmod(c8, 2)
                    nc.tensor.matmul(
                        out=ps[:, k, :],
                        lhsT=corner[(g, j)][:, E * xb + 128 * chk:
                                            E * xb + 128 * (chk + 1)],
                        rhs=diag[(g, c8)][:],
                        start=(c8 == 0), stop=(c8 == 7))
            for k in range(3):
                chk = 3 * half + k
                ops.append(nc.vector.tensor_copy(
                    out=sampT[chk][:, 128 * g:128 * (g + 1)],
                    in_=ps[:, k, :]))
        return ops

    def u_mms():
        u_sb = []
        for co in range(CH):
            ps = psum([128, BPC * NH])
            for ci in range(CH):
                nc.tensor.matmul(out=ps[:],
                                 lhsT=w_t["wkn"][:, ci, 128 * co:128 * (co + 1)],
                                 rhs=qm[ci][:], start=(ci == 0),
                                 stop=(ci == CH - 1))
            t = ctile([128, BPC * NH], BF16, tag=f"u{co}")
            nc.vector.tensor_copy(out=t[:], in_=ps[:])
            u_sb.append(t)
        return u_sb

    # ---- per-wave attention (v-route) ----
    ctxT = ctile([128, CH, BPC], BF16, tag="ctxT")
    outfin = [ctile([4, E], BF16, tag=f"of{g}") for g in range(NG)]
    v_ps = {}
    at12 = [tpool.tile([NH, 4, NB], BF16, tag="at", name=f"at{g}")
            for g in range(NG)]
    abc_ps = {}

    def score_mms(g):
        sc_ps = psum([NH, 4, NB])
        for bl in range(4):
            b = 4 * g + bl
            for ci in range(CH):
                nc.tensor.matmul(
                    out=sc_ps[:, bl, :],
                    lhsT=u_sb[ci][:, NH * b:NH * (b + 1)],
                    rhs=sampT[ci][:, 128 * g + NB * bl:128 * g + NB * (bl + 1)],
                    start=(ci == 0), stop=(ci == CH - 1))
        return sc_ps

    def softmax(g, sc_ps):
        # no max-sub; logits are small by construction
        ex = tpool.tile([NH, 4, NB], F32, tag="ex", name=f"ex{g}")
        nc.scalar.activation(out=ex[:], in_=sc_ps[:], func=ACTF.Exp)
        sm = tpool.tile([NH, 4, 1], F32, tag="sm", name=f"sm{g}")
        nc.vector.tensor_reduce(out=sm[:], in_=ex[:], axis=AX.X, op=ALU.add)
        rc = tpool.tile([NH, 4, 1], F32, tag="rc", name=f"rc{g}")
        nc.vector.reciprocal(out=rc[:], in_=sm[:])
        nc.vector.tensor_mul(out=at12[g][:], in0=ex[:],
                             in1=rc[:].to_broadcast([NH, 4, NB]))

    def v_mms(g):
        for half in range(2):
            ps = abcp.tile([128, 3, 128], F32, tag="vps", name=f"v{g}_{half}")
            for k in range(3):
                co = 3 * half + k
                for ci in range(CH):
                    nc.tensor.matmul(
                        out=ps[:, k, :],
                        lhsT=w_t["wvt"][:, ci, 128 * co:128 * (co + 1)],
                        rhs=sampT[ci][:, 128 * g:128 * (g + 1)],
                        start=(ci == 0), stop=(ci == CH - 1))
            v_ps[(g, half)] = ps

    def bsel_mms(g):
        # abc[c, (bl, p)] = attn[4g+bl, head(c), p], via PE broadcast.
        # walrus allows only one PSUM input per DVE op, so abc moves to
        # SBUF (bf16) before the ctx multiply; v stays in PSUM.
        for half in range(2):
            ps = abcp.tile([128, 3, 4 * NB], F32, tag="abc",
                           name=f"abc{g}_{half}")
            for k in range(3):
                nc.tensor.matmul(
                    out=ps[:, k, :],
                    lhsT=cb[0:NH, CB_BSEL + 128 * (3 * half + k):
                            CB_BSEL + 128 * (3 * half + k + 1)],
                    rhs=at12[g][:], start=True, stop=True)
            sb = tpool.tile([128, 3, 4 * NB], BF16, tag="absb",
                            name=f"absb{g}_{half}")
            nc.scalar.copy(out=sb[:], in_=ps[:])
            abc_ps[(g, half)] = sb

    def ctx_sel(g):
        # half-major so ctxT chunks 0-2 are ready before v/abc half 1 lands
        prod = tpool.tile([128, CH, 4, NB], F32, tag="prod", name=f"prod{g}")
        for vh in range(2):
            for k in range(3):
                chk = 3 * vh + k
                nc.vector.tensor_mul(
                    out=prod[:, chk],
                    in0=v_ps[(g, vh)][:, k, :]
                        .rearrange("p (b n) -> p b n", n=NB),
                    in1=abc_ps[(g, vh)][:, k, :]
                        .rearrange("p (b n) -> p b n", n=NB))
            with nc.allow_low_precision(reason="32-term attn sum; bf16 out "
                                        "is within error budget"):
                nc.vector.tensor_reduce(
                    out=ctxT[:, 3 * vh:3 * (vh + 1), 4 * g:4 * (g + 1)],
                    in_=prod[:, 3 * vh:3 * (vh + 1)], axis=AX.X, op=ALU.add)

    def outproj(g):
        # ci-interleaved across the two E-halves so the accumulation
        # pipelines with ctxT chunk arrivals; the out bias enters the psum
        # as a rank-1 ones x bias matmul (bsel row 0 is all-ones over the
        # first 4 columns)
        pss = [psum([4, 384]) for _ in range(2)]
        for half in range(2):
            nc.tensor.matmul(out=pss[half][:],
                             lhsT=cb[0:1, CB_BSEL:CB_BSEL + 4],
                             rhs=bo_t[0:1, E * g + 384 * half:
                                      E * g + 384 * (half + 1)],
                             start=True, stop=False)
        for ci in range(CH):
            for half in range(2):
                nc.tensor.matmul(out=pss[half][:],
                                 lhsT=ctxT[:, ci, 4 * g:4 * (g + 1)],
                                 rhs=w_t["wot"][:, ci,
                                               384 * half:384 * (half + 1)],
                                 start=False, stop=(ci == CH - 1))
        nc.scalar.mul(out=outfin[g][0:4, 0:384], in_=pss[0][:],
                      mul=cf[0:4, CF_CONF + g:CF_CONF + g + 1])
        nc.vector.tensor_scalar(out=outfin[g][0:4, 384:768], in0=pss[1][:],
                                scalar1=cf[0:4, CF_CONF + g:CF_CONF + g + 1],
                                scalar2=None, op0=ALU.mult)

    def write_batch(b):
        g, bl = divmod(b, 4)
        bt = btpool.tile([128, E], F32, tag="bt", name=f"bt{b}")
        for half in range(2):
            sl = slice(384 * half, 384 * (half + 1))
            ps = psum([128, 384])
            nc.tensor.matmul(
                out=ps[:],
                lhsT=cb[0:4, CB_ONEH + 128 * bl:CB_ONEH + 128 * (bl + 1)],
                rhs=outfin[g][:, sl], start=True, stop=True)
            if half == 0:
                nc.scalar.copy(out=bt[:, sl], in_=ps[:])
            else:
                nc.vector.tensor_copy(out=bt[:, sl], in_=ps[:])
        r0 = FULLN * b
        nc.sync.dma_start(
            out=out[r0:r0 + 512, :].rearrange("(i p) c -> p i c", p=128),
            in_=bt[:].unsqueeze(1).to_broadcast([128, 4, E]))
        nc.sync.dma_start(out=out[r0 + 512:r0 + 513, :], in_=bt[0:1, :])

    # emission order follows expected data arrival (each engine's stream is
    # executed in order; a stalled instruction blocks everything behind it).
    # Target DMA-queue order: consts, wq, wk, g0 gathers, wv, wo, g1
    # gathers, then output writes back-to-back.
    from concourse.tile_rust import add_dep_helper

    # wv/wo enter the DMA queue after the g0 gathers (dep fires ~when q is
    # done) but before the g1 gathers (deferred further below); chunked so
    # the consuming matmuls pipeline with chunk arrivals
    for name, ap in (("wvt", wvt), ("wot", wot)):
        t = cpool.tile([128, CH, E], BF16, tag=name)
        for c2 in range(CH // 2):
            d = nc.sync.dma_start(out=t[:, 2 * c2:2 * c2 + 2, :],
                                  in_=ap[:, 2 * c2:2 * c2 + 2, :])
            add_dep_helper(d.ins, qT_ops[5].ins,
                           reason="defer weight load behind g0 gathers")
        w_t[name] = t

    diag_mms(1)
    tr0 = tr_mms(0)
    u_sb = u_mms()
    for j in range(4):
        add_dep_helper(gather_dmas[(1, j)].ins, tr0[0].ins,
                       reason="defer g1 gathers behind wv/wo weight loads")
    sc0 = score_mms(0)
    softmax(0, sc0)
    v_mms(0)
    bsel_mms(0)
    ctx_sel(0)
    outproj(0)
    for b in range(4):
        write_batch(b)
    tr_mms(1)
    sc1 = score_mms(1)
    softmax(1, sc1)
    v_mms(1)
    bsel_mms(1)
    ctx_sel(1)
    outproj(1)
    for b in range(4, 8):
        write_batch(b)


_NO_SPLIT_TYPES = {"InstUnconditionalBranch", "InstConditionalBranch"}


def _split_waits(nc, max_waits=1):
    # walrus (CoreV3) accepts only one sync-wait command per compute
    # instruction; move extra waits onto injected same-engine NoOps placed
    # immediately before the instruction (semantics unchanged).
    import bass_rust
    k = 0
    for fn in nc.m.functions:
        for bb in fn.blocks:
            insts = bb.instructions
            i = 0
            while i < len(insts):
                inst = insts[i]
                si = inst.sync_info
                if (type(inst).__name__ not in _NO_SPLIT_TYPES
                        and si is not None
                        and si.on_wait and len(si.on_wait) > max_waits):
                    waits = list(si.on_wait)
                    extra, keep = waits[:-max_waits], waits[-max_waits:]
                    for w in extra:
                        k += 1
                        nop = bass_rust.InstNoOp(name=f"I-wsplit-{k}",
                                                 engine=inst.engine,
                                                 ins=[], outs=[])
                        nop.sync_info = bass_rust.SyncInfo(on_wait=[w],
                                                           on_update=[])
                        insts.insert(i, nop)
                        i += 1
                    inst.sync_info = bass_rust.SyncInfo(
                        on_wait=keep, on_update=list(si.on_update or []))
                i += 1
    return k


def build(split=True):
    from contextlib import ExitStack

    nc = bass.Bass("TRN2", debug=False, num_devices=NCORES)
    with tile.TileContext(nc) as tc, ExitStack() as es:
        _body(es, tc)
    if split:
        # needed for the walrus compile; CoreSim can't replay injected nops
        _split_waits(nc)
    return nc


def host_prep(inputs):
    """Build per-core in_maps from full inputs (layout/dtype marshalling +
    weights-only fusion)."""
    BF = ml_dtypes.bfloat16
    x = np.asarray(inputs["x"], dtype=np.float32)
    bio = np.asarray(inputs["bio_embed"], dtype=np.float32)
    base = np.asarray(inputs["base_coords"], dtype=np.float32)
    offsets = np.asarray(inputs["offsets"], dtype=np.float32)
    confidence = np.asarray(inputs["confidence"], dtype=np.float32)
    wsp = np.asarray(inputs["sample_proj_w"], dtype=np.float32)
    bsp = np.asarray(inputs["sample_proj_b"], dtype=np.float32)
    win = np.asarray(inputs["in_proj_w"], dtype=np.float32)
    bin_ = np.asarray(inputs["in_proj_b"], dtype=np.float32)
    wout = np.asarray(inputs["out_proj_w"], dtype=np.float32)
    bout = np.asarray(inputs["out_proj_b"], dtype=np.float32)

    # weights-only fusion
    wq = win[:E]
    wkp = win[E:2 * E] @ wsp
    wvp = win[2 * E:] @ wsp
    bop = wout @ (win[2 * E:] @ bsp + bin_[2 * E:]) + bout

    F8 = ml_dtypes.float8_e4m3

    def chunkT(w, dt=BF):  # [E, E] -> [128, CH, E] of w^T
        return np.ascontiguousarray(
            w.T.reshape(CH, 128, E).transpose(1, 0, 2)).astype(dt)

    def chunkN(w, dt=BF):  # [E, E] -> [128, CH, E] natural rows
        return np.ascontiguousarray(
            w.reshape(CH, 128, E).transpose(1, 0, 2)).astype(dt)

    consF = np.zeros((128, CF_W), np.float32)
    consF[:, CF_BASE:CF_BASE + 6] = np.tile(np.tile(base, (4, 1)), (1, 2)) \
        .reshape(128, 6)
    consF[:, CF_ROWB:CF_ROWB + 2] = (
        (np.arange(ROWS) // NB) * FULLN + 1.0).astype(np.float32) \
        .reshape(2, 128).T
    consF[:, CF_MUL3:CF_MUL3 + 6] = np.tile(
        np.array([1.0, 8.0, 64.0], np.float32), (128, 2))
    consF[:, CF_BQ:CF_BQ + CH] = (bin_[:E] * 0.125).reshape(CH, 128).T
    hsel = np.zeros((128, CH, NH), np.float32)
    for ci in range(CH):
        for p in range(128):
            hsel[p, ci, (ci * 128 + p) // HD] = 1.0
    consF[:, CF_HSEL:CF_HSEL + CH * NH] = hsel.reshape(128, CH * NH)

    consB = np.zeros((128, CB_W), np.float32)
    consB[:, CB_IDEN:CB_IDEN + 128] = np.eye(128, dtype=np.float32)
    for c in range(E):
        consB[c // HD, CB_BSEL + c] = 1.0
    for j in range(4):
        consB[j, CB_ONEH + 128 * j:CB_ONEH + 128 * (j + 1)] = 1.0

    shared = {
        "wqt": chunkT(wq, F8),
        "wkn": chunkN(wkp, F8),
        "wvt": chunkT(wvp),
        "wot": chunkT(wout),
    }
    bo_full = np.tile(bop, NG)[None, :].astype(BF)

    x16 = x.astype(BF)
    in_maps = []
    for c in range(NCORES):
        bsl = slice(BPC * c, BPC * (c + 1))
        cF = consF.copy()
        # offsets per row: row r of group g = batch 4g + r//32, point r%32
        offs_c = offsets[bsl].reshape(NG, 128, 3)
        cF[:, CF_OFFS:CF_OFFS + 3] = offs_c[0]
        cF[:, CF_OFFS + 3:CF_OFFS + 6] = offs_c[1]
        cF[0:4, CF_CONF:CF_CONF + NG] = confidence[bsl, 0].reshape(NG, 4).T
        cB = consB.copy()
        cB[:, CB_BIO:CB_BIO + CH * BPC] = np.ascontiguousarray(
            bio[bsl].T.reshape(CH, 128, BPC).transpose(1, 0, 2)) \
            .reshape(128, CH * BPC)
        m = dict(shared)
        m["x"] = x16[bsl].reshape(BPC * FULLN, E)
        m["consF"] = cF
        m["consB"] = cB.astype(BF)
        m["bo_bc"] = bo_full
        in_maps.append(m)
    return in_maps


_NC = None


def kernel(**inputs):
    global _NC
    if _NC is None:
        _NC = build()
    in_maps = host_prep(inputs)
    res = bass_utils.run_bass_kernel_spmd(_NC, in_maps,
                                          core_ids=list(range(NCORES)))
    outs = [res.results[c]["out"].reshape(BPC, FULLN, E)
            for c in range(NCORES)]
    return np.concatenate(outs, axis=0)
